# revision 51
# baseline (speedup 1.0000x reference)
"""BoxConv2d Trainium2 kernel (8 NeuronCores, SPMD).

Math: the reference computes, per output channel k = (c, f),
    out[b,k] = interp-row(I) diff, then interp-col diff
where I is the zero-padded integral image of input[b,c].  That whole
pipeline (integral image + fractional box-edge interpolation) is linear
in the input and separable, so it collapses to two dense 128x128
matrix products per image:

    out[b,k] = A_k @ x[b,c] @ B_k^T

with banded "pixel overlap" matrices
    A_k[xo, a] = clamp(xo - a + x_max_k + 1, 0, 1)
                 - clamp(xo - a + x_min_k, 0, 1)
(the overlap length between the box row extent [xo+x_min, xo+x_max+1]
and the pixel row [a, a+1]), and likewise B_k for columns.  A/B are
built on the host from the tiny (C,F) box params; the device does pure
128-contraction matmuls on the PE array.

Sharding: the K = C*F = 128 output channels are split across 8 cores
(16 channels = 4 in_planes per core), so each core reads only its own
4 input planes and input reads are not duplicated chip-wide.

Device dataflow per core:
  pass 1 (per b,c):     V[j, (f,xo)]  = x_bc^T A^T  (lhsT=x_bc, N=512)
  pass 2 (per c,f,b/2): O[yo, (b,xo)] = B_k V       (lhsT=B_k^T, N=512)
Both passes stream 512 columns per matmul so float32r runs at the full
PE rate (plain float32 matmul costs 4 cycles/column).  Pass 2 emits the
output transposed (yo on partitions); it is stored transposed in DRAM
as one fully-contiguous 256KB block per (kl, batch-half) and the host
untransposes while assembling.  PSUM->SBUF copies are split across the
Scalar (V) and Vector (O) engines; V lives in per-half-batch tiles so
each pass-2 half only waits on 4 copies, and the c-loop is software-
pipelined at half-batch granularity so the PE never idles on copies
and the 8.4MB/core output stream starts as early as possible (the DMA
engines are this kernel's saturated resource).

Numerics: float32r multiplies at reduced (~tf32) precision; measured
l2 relative error vs the fp32 reference is ~1.5e-4 (max abs err ~2e-4
of the output scale).  Set BOXCONV_MM_DT=f32 for full fp32 matmuls
(~1.4x slower end-to-end, rel err ~7e-7).
"""

import os
import sys

if "/opt/trn_rl_repo" not in sys.path:
    sys.path.insert(0, "/opt/trn_rl_repo")

import numpy as np

import concourse.bass as bass  # noqa: F401
import concourse.mybir as mybir
import concourse.tile as tile
from concourse import bacc
from concourse.bass_utils import run_bass_kernel_spmd

B, C, F, H, W = 8, 32, 4, 128, 128
NCORES = 8
CPC = C // NCORES  # in_planes per core
KPC = CPC * F      # output channels per core
BH = B // 2        # batch half

_DT = mybir.dt.float32
_MM_DT = {
    "f32": mybir.dt.float32,
    "f32r": mybir.dt.float32r,
}[os.environ.get("BOXCONV_MM_DT", "f32r")]

_NC_CACHE = {}
LAST_RESULT = None


def _build_nc():
    nc = bacc.Bacc(
        "TRN2", target_bir_lowering=False, debug=False, num_devices=NCORES
    )
    x_p = nc.declare_dram_parameter("x", [B, H, CPC * W], _MM_DT, isOutput=False)
    at_p = nc.declare_dram_parameter(
        "at", [CPC, H, F * H], _MM_DT, isOutput=False)
    bt_p = nc.declare_dram_parameter(
        "bt", [CPC, W, F * W], _MM_DT, isOutput=False)
    # transposed output, one contiguous 256KB block per (kl, half):
    # outT[kl, h, yo, (bh, xo)] = out[b=h*4+bh, kl, xo, yo]
    out_p = nc.declare_dram_parameter(
        "outT", [KPC, 2, W, BH * H], _DT, isOutput=True)

    with tile.TileContext(nc) as tc:
        with (
            tc.tile_pool(name="const", bufs=1) as cpool,
            tc.tile_pool(name="xin", bufs=B) as xpool,
            tc.tile_pool(name="vall", bufs=6) as vpool,
            tc.tile_pool(name="osb", bufs=6) as opool,
            tc.tile_pool(name="pv", bufs=4, space="PSUM") as pvpool,
            tc.tile_pool(name="po", bufs=4, space="PSUM") as popool,
        ):
            at_sb = [None] * CPC
            bt_sb = [None] * CPC
            x_sb = [None] * B

            def load_at(c):
                at_sb[c] = cpool.tile(
                    [128, F * H], _MM_DT, name=f"at{c}", tag=f"at{c}"
                )
                nc.sync.dma_start(at_sb[c][:], at_p[c])

            def load_bt(c):
                bt_sb[c] = cpool.tile(
                    [128, F * W], _MM_DT, name=f"bt{c}", tag=f"bt{c}"
                )
                nc.sync.dma_start(bt_sb[c][:], bt_p[c])

            def load_x(b):
                x_sb[b] = xpool.tile(
                    [128, CPC * W], _MM_DT, name=f"xsb{b}", tag="x"
                )
                nc.sync.dma_start(x_sb[b][:], x_p[b])

            # order loads so pass1(c=0) starts early AND runs gapless:
            # x1 lands before the first matmul issues, so MMs 0-3 are
            # back-to-back and the PE HAM clock-gate warms immediately
            load_x(0)
            load_x(1)
            load_at(0)
            for b in range(2, B):
                load_x(b)
            load_bt(0)
            load_at(1)
            load_bt(1)
            load_at(2)
            load_bt(2)
            load_at(3)
            load_bt(3)

            # V is held in per-half-batch tiles so pass 2 of a half only
            # depends on that half's 4 PSUM->SBUF copies (tile-granular
            # dependency tracking), starting the output stream earlier.
            v_half = [[None] * 2 for _ in range(CPC)]

            def emit_pass1(c, h):
                # V_h[j, (f, bh, xo)], bh = b - 4h
                vt = vpool.tile([128, F * BH * H], _MM_DT,
                                name=f"vall{c}{h}", tag="vall")
                v_half[c][h] = vt
                v_r = vt[:].rearrange("p (f bh xo) -> p f bh xo", f=F, bh=BH)
                for bh in range(BH):
                    b = h * BH + bh
                    # V[j, (f,xo)] = sum_a x[a, j] * A_k[xo, a]
                    v_ps = pvpool.tile([128, F * H], mybir.dt.float32,
                                       name=f"vps{c}{b}", tag="vps")
                    nc.tensor.matmul(
                        v_ps[:],
                        lhsT=x_sb[b][:, c * W:(c + 1) * W],
                        rhs=at_sb[c][:],
                        start=True,
                        stop=True,
                    )
                    # scatter the 4 f-blocks into V_h's (f, bh, .) slots
                    nc.vector.tensor_copy(v_r[:, :, bh, :], v_ps[:])

            def emit_pass2(c, h):
                vt = v_half[c][h]
                for f in range(F):
                    kl = c * F + f
                    # O[yo, (bh,xo)] = sum_j B_k[yo,j] * V[j, (bh,xo)]
                    o_ps = popool.tile([128, BH * H], mybir.dt.float32,
                                       name=f"ops{c}{f}{h}", tag="ops")
                    nc.tensor.matmul(
                        o_ps[:],
                        lhsT=bt_sb[c][:, f * W:(f + 1) * W],
                        rhs=vt[:, f * BH * H:(f + 1) * BH * H],
                        start=True,
                        stop=True,
                    )
                    o_sb = opool.tile([128, BH * H], _DT,
                                      name=f"osb{c}{f}{h}", tag="osb")
                    nc.scalar.copy(o_sb[:], o_ps[:])
                    # one fully-contiguous 256KB DRAM write
                    nc.sync.dma_start(out_p[kl, h], o_sb[:])

            # software pipeline at half-batch granularity: each pass-2
            # half runs one pass-1 half after its V copies were issued,
            # keeping the PE dense and the DRAM outflow smooth
            emit_pass1(0, 0)
            emit_pass1(0, 1)
            for c in range(1, CPC):
                emit_pass2(c - 1, 0)
                emit_pass1(c, 0)
                emit_pass2(c - 1, 1)
                emit_pass1(c, 1)
            emit_pass2(CPC - 1, 0)
            emit_pass2(CPC - 1, 1)
    nc.finalize()
    return nc


def _get_nc():
    if "nc" not in _NC_CACHE:
        _NC_CACHE["nc"] = _build_nc()
    return _NC_CACHE["nc"]


def _overlap_mats(lo, hi):
    """(K, out, in) pixel-overlap matrices for a 128-wide axis."""
    t = np.arange(128, dtype=np.float64)
    d = t[:, None] - t[None, :]  # out - in
    lo = lo.astype(np.float64)[:, None, None]
    hi = hi.astype(np.float64)[:, None, None]
    m = np.clip(d[None] + hi + 1.0, 0.0, 1.0) - np.clip(d[None] + lo, 0.0, 1.0)
    return m.astype(np.float32)


def _make_in_maps(input, x_min, x_max, y_min, y_max):
    A = _overlap_mats(x_min.reshape(-1), x_max.reshape(-1))   # (K, xo, a)
    Bm = _overlap_mats(y_min.reshape(-1), y_max.reshape(-1))  # (K, yo, j)
    in_maps = []
    for m in range(NCORES):
        cs = slice(CPC * m, CPC * (m + 1))
        ks = slice(KPC * m, KPC * (m + 1))
        xm = input[:, cs].transpose(0, 2, 1, 3)
        xm = xm.reshape(B, H, CPC * W)                        # [b, a, (c, j)]
        # at[c, a, (f, xo)] = A[k=c*F+f, xo, a]
        at = A[ks].reshape(CPC, F, H, H).transpose(0, 3, 1, 2)
        bt = Bm[ks].reshape(CPC, F, W, W).transpose(0, 3, 1, 2)
        in_maps.append({
            "x": np.ascontiguousarray(xm, dtype=np.float32),
            "at": np.ascontiguousarray(
                at.reshape(CPC, H, F * H), dtype=np.float32),
            "bt": np.ascontiguousarray(
                bt.reshape(CPC, W, F * W), dtype=np.float32),
        })
    return in_maps


def _assemble(results):
    out = np.empty((B, C * F, H, W), np.float32)
    for m in range(NCORES):
        # outT[kl, h, yo, bh, xo] -> out[b=h*4+bh, kl, xo, yo]
        o = results[m]["outT"].reshape(KPC, 2, W, BH, H)
        o = o.transpose(1, 3, 0, 4, 2).reshape(B, KPC, H, W)
        out[:, KPC * m:KPC * (m + 1)] = o
    return out


def _run(inputs, trace=False):
    global LAST_RESULT
    nc = _get_nc()
    in_maps = _make_in_maps(**inputs)
    LAST_RESULT = run_bass_kernel_spmd(
        nc, in_maps, list(range(NCORES)), trace=trace
    )
    return _assemble(LAST_RESULT.results)


def kernel(input, x_min, x_max, y_min, y_max):
    return _run({
        "input": np.asarray(input, dtype=np.float32),
        "x_min": np.asarray(x_min, dtype=np.float32),
        "x_max": np.asarray(x_max, dtype=np.float32),
        "y_min": np.asarray(y_min, dtype=np.float32),
        "y_max": np.asarray(y_max, dtype=np.float32),
    })
